# revision 7
# baseline (speedup 1.0000x reference)
"""Trainium2 Bass kernel for nn_AttentionLayer_77524159693050 (retrieval_knn).

Math reduction (verified against the reference):
  e[n,k] = eX[n] + eC[n,k]; top_k with k=KC-1 then sort == drop argmin_k eC.
  eC[n,k] = Candidate[n,k,:]@v1 + t[neigh_ind[n,k]],  t = data_m_train @ v2,
  where v = W @ A[OUT:,0], v1 = v[:DIM], v2 = v[DIM:].
  a_out/b_out only need s[j] = sum_d data_m_train[j,d] at neigh_ind positions.

Device program 1 (SPMD x8, data_m_train row-sharded): (t, s) pair tables.
Host glue: table lookup ts_table[neigh_ind] (the HW indirect DMA is
one-index-per-partition, so fine-grained gather is done host-side), plus
final index-selection of the small nd/ni outputs using the device argmin.
Device program 2 (SPMD x8, n row-sharded): e scores, argmin, and the heavy
10-of-11 candidate row selection (225MB read + 205MB write), spread across
the Vector, GpSimd and Scalar engines.
"""

import sys
import types
import numpy as np

sys.path.insert(0, "/opt/trn_rl_repo")

from concourse import bass, mybir, bacc  # noqa: E402
from concourse.tile import TileContext  # noqa: E402
from concourse.bass_utils import run_bass_kernel_spmd  # noqa: E402

f32 = mybir.dt.float32
i32 = mybir.dt.int32

NO, KC, DIM, NT, OUT, K = 20000, 11, 256, 100000, 128, 10
NCORES = 8
NO_SH = NO // NCORES          # 2500
NO_PAD = 2560                 # 20 tiles of 128
NTILE = NO_PAD // 128         # 20
NT_SH = NT // NCORES          # 12500
NT_BLK = 98                   # 128-row blocks per core
NT_PAD = 128 * NT_BLK         # 12544
P = 128

TRACE = {"enabled": False, "exec_ns": []}

_cache = {}


def _ntff_hook():
    try:
        from trn_agent_boot import trn_boot
        hook = trn_boot._ntff_profile_via_ctypes("/opt/axon/libaxon_pjrt.so")
        mod = types.ModuleType("antenv.axon_hooks")
        mod.get_axon_ntff_profile_hook = lambda: hook
        sys.modules["antenv.axon_hooks"] = mod
        return True
    except Exception:
        return False


def build_p1():
    nc = bacc.Bacc()
    dmt_d = nc.declare_dram_parameter("dmt", [NT_PAD, DIM], f32, isOutput=False)
    v2r_d = nc.declare_dram_parameter("v2r", [P, DIM], f32, isOutput=False)
    ones_d = nc.declare_dram_parameter("ones", [P, DIM], f32, isOutput=False)
    tsloc_d = nc.declare_dram_parameter("tsloc", [NT_PAD, 2], f32, isOutput=True)

    GB = 14            # 256-col blocks per DMA group
    NG = NT_BLK // GB  # 7
    NACT = 10          # s-blocks per group handled by ACT (rest on DVE)

    # partition-major: partition p holds shard rows [p*98, (p+1)*98)
    dmt_v = dmt_d[:].rearrange("(p i) d -> p i d", i=NT_BLK)      # [128, 98, 256]
    tsloc_v = tsloc_d[:].rearrange("(p i) c -> p i c", i=NT_BLK)  # [128, 98, 2]
    mult, add = mybir.AluOpType.mult, mybir.AluOpType.add

    with TileContext(nc) as tc:
        with (
            tc.tile_pool(name="io", bufs=3) as io_pool,
            tc.tile_pool(name="scr", bufs=2) as scr_pool,
            tc.tile_pool(name="acc", bufs=1) as acc_pool,
        ):
            v2r_t = acc_pool.tile([P, DIM], f32)
            nc.sync.dma_start(out=v2r_t[:], in_=v2r_d[:])
            ones_t = acc_pool.tile([P, DIM], f32)
            nc.sync.dma_start(out=ones_t[:], in_=ones_d[:])
            tsacc = acc_pool.tile([P, NT_BLK, 2], f32)
            for g in range(NG):
                in_t = io_pool.tile([P, GB, DIM], f32, tag="in")
                nc.sync.dma_start(out=in_t[:], in_=dmt_v[:, g * GB:(g + 1) * GB, :])
                for b in range(GB):
                    i = g * GB + b
                    scr_t = scr_pool.tile([P, DIM], f32, tag="scr")
                    nc.vector.scalar_tensor_tensor(
                        out=scr_t[:], in0=in_t[:, b, :], scalar=0.0,
                        in1=v2r_t[:], op0=add, op1=mult,
                        accum_out=tsacc[:, i, 0:1])
                    if b < NACT:
                        ascr_t = scr_pool.tile([P, DIM], f32, tag="ascr")
                        nc.scalar.activation(
                            out=ascr_t[:], in_=in_t[:, b, :],
                            func=mybir.ActivationFunctionType.Copy,
                            accum_out=tsacc[:, i, 1:2])
                    else:
                        sscr_t = scr_pool.tile([P, DIM], f32, tag="sscr")
                        nc.vector.scalar_tensor_tensor(
                            out=sscr_t[:], in0=in_t[:, b, :], scalar=0.0,
                            in1=ones_t[:], op0=add, op1=mult,
                            accum_out=tsacc[:, i, 1:2])
            nc.sync.dma_start(out=tsloc_v, in_=tsacc[:])
    nc.compile()
    return nc


def build_p2():
    nc = bacc.Bacc()
    cand_d = nc.declare_dram_parameter("cand", [NO_PAD, KC, DIM], f32, isOutput=False)
    pk_d = nc.declare_dram_parameter("pk", [NO_PAD, KC], f32, isOutput=False)
    v1r_d = nc.declare_dram_parameter("v1r", [P, DIM], f32, isOutput=False)
    oc_d = nc.declare_dram_parameter("oc", [NO_PAD, K, DIM], f32, isOutput=True)
    om_d = nc.declare_dram_parameter("om", [P, NTILE], f32, isOutput=True)

    mult, add, sub = mybir.AluOpType.mult, mybir.AluOpType.add, mybir.AluOpType.subtract
    is_eq, is_lt = mybir.AluOpType.is_equal, mybir.AluOpType.is_lt

    with TileContext(nc) as tc:
        with (
            tc.tile_pool(name="const", bufs=1) as cpool,
            tc.tile_pool(name="cio", bufs=4) as cio,
            tc.tile_pool(name="oio", bufs=4) as oio,
            tc.tile_pool(name="work", bufs=3) as wp,
            tc.tile_pool(name="small", bufs=4) as sp,
        ):
            v1r_t = cpool.tile([P, DIM], f32)
            nc.sync.dma_start(out=v1r_t[:], in_=v1r_d[:])
            kio_i = cpool.tile([P, KC], i32)
            nc.gpsimd.iota(kio_i[:], pattern=[[1, KC]], base=0, channel_multiplier=0)
            kio_f = cpool.tile([P, KC], f32)
            nc.vector.tensor_copy(kio_f[:], kio_i[:])
            mst_t = cpool.tile([P, NTILE], f32)

            for t in range(NTILE):
                r0 = t * P
                cand_t = cio.tile([P, KC, DIM], f32, tag="cand")
                nc.sync.dma_start(out=cand_t[:], in_=cand_d[r0:r0 + P])
                pk_t = sp.tile([P, KC], f32, tag="pk")
                nc.sync.dma_start(out=pk_t[:], in_=pk_d[r0:r0 + P])

                # e[p,k] = t_k + sum_d cand*v1  (c via fused mul+accum per k)
                c_t = sp.tile([P, KC], f32, tag="c")
                for k in range(KC):
                    scr_t = wp.tile([P, DIM], f32, tag="scr")
                    nc.vector.scalar_tensor_tensor(
                        out=scr_t[:], in0=cand_t[:, k, :], scalar=0.0,
                        in1=v1r_t[:], op0=add, op1=mult,
                        accum_out=c_t[:, k:k + 1])
                e_t = sp.tile([P, KC], f32, tag="e")
                nc.gpsimd.tensor_tensor(out=e_t[:], in0=c_t[:], in1=pk_t[:], op=add)

                # argmin over k
                mn_t = sp.tile([P, 1], f32, tag="mn")
                nc.vector.tensor_reduce(out=mn_t[:], in_=e_t[:],
                                        axis=mybir.AxisListType.X,
                                        op=mybir.AluOpType.min)
                scr11_t = sp.tile([P, KC], f32, tag="scr11")
                m_t = sp.tile([P, 1], f32, tag="m")
                nc.vector.scalar_tensor_tensor(
                    out=scr11_t[:], in0=e_t[:], scalar=mn_t[:], in1=kio_f[:],
                    op0=is_eq, op1=mult, accum_out=m_t[:])
                nc.scalar.activation(out=mst_t[:, t:t + 1], in_=m_t[:],
                                     func=mybir.ActivationFunctionType.Copy)
                mask_t = sp.tile([P, K], f32, tag="mask")
                nc.gpsimd.tensor_scalar(out=mask_t[:], in0=kio_f[:, 0:K],
                                        scalar1=m_t[:], scalar2=None, op0=is_lt)

                invm_t = sp.tile([P, K], f32, tag="invm")
                nc.scalar.activation(out=invm_t[:], in_=mask_t[:],
                                     func=mybir.ActivationFunctionType.Copy,
                                     scale=-1.0, bias=1.0)

                # selection: out_j = mask_j*A_j + (1-mask_j)*B_j
                mm_t = wp.tile([P, K, DIM], f32, tag="mm")
                for j in range(K):
                    nc.scalar.activation(out=mm_t[:, j, :], in_=cand_t[:, j + 1, :],
                                         func=mybir.ActivationFunctionType.Copy,
                                         scale=invm_t[:, j:j + 1])
                out_t = oio.tile([P, K, DIM], f32, tag="oc")
                for j in range(K):
                    if j < 6:
                        nc.vector.scalar_tensor_tensor(
                            out=out_t[:, j, :], in0=cand_t[:, j, :],
                            scalar=mask_t[:, j:j + 1], in1=mm_t[:, j, :],
                            op0=mult, op1=add)
                    else:
                        ga_t = wp.tile([P, DIM], f32, tag="ga")
                        nc.gpsimd.tensor_scalar(
                            out=ga_t[:], in0=cand_t[:, j, :],
                            scalar1=mask_t[:, j:j + 1], scalar2=None, op0=mult)
                        nc.gpsimd.tensor_tensor(
                            out=out_t[:, j, :], in0=ga_t[:], in1=mm_t[:, j, :],
                            op=add)
                nc.sync.dma_start(out=oc_d[r0:r0 + P], in_=out_t[:])

            nc.sync.dma_start(out=om_d[:], in_=mst_t[:])
    nc.compile()
    return nc


def _run(nc, in_maps, tag):
    if TRACE["enabled"]:
        import shutil
        _ntff_hook()
        shutil.rmtree(f"/tmp/knn_trace_{tag}", ignore_errors=True)
        res = run_bass_kernel_spmd(nc, in_maps, core_ids=list(range(NCORES)),
                                   trace=True, tmpdir=f"/tmp/knn_trace_{tag}")
        TRACE["exec_ns"].append((tag, res.exec_time_ns))
        return res
    return run_bass_kernel_spmd(nc, in_maps, core_ids=list(range(NCORES)))


def kernel(X, Candidate, neigh_dist, neigh_ind, data_m_train, data_m_batch,
           test, W, A, **_unused):
    Candidate = np.ascontiguousarray(np.asarray(Candidate, dtype=np.float32))
    neigh_dist = np.ascontiguousarray(np.asarray(neigh_dist, dtype=np.float32))
    ni_in = np.asarray(neigh_ind)
    ni = ni_in.astype(np.int64)
    dmt = np.ascontiguousarray(np.asarray(data_m_train, dtype=np.float32))
    W = np.asarray(W, dtype=np.float32)
    A = np.asarray(A, dtype=np.float32)

    v = (W.astype(np.float64) @ A[OUT:, 0].astype(np.float64)).astype(np.float32)
    v1, v2 = v[:DIM], v[DIM:]
    v1r = np.ascontiguousarray(np.broadcast_to(v1, (P, DIM)))
    v2r = np.ascontiguousarray(np.broadcast_to(v2, (P, DIM)))

    # ---- program 1: (t, s) pair tables ----
    if "p1" not in _cache:
        _cache["p1"] = build_p1()
    dmt_pad = np.zeros((NCORES * NT_PAD, DIM), np.float32)
    for c in range(NCORES):
        dmt_pad[c * NT_PAD:c * NT_PAD + NT_SH] = dmt[c * NT_SH:(c + 1) * NT_SH]
    onesr = np.ones((P, DIM), np.float32)
    in1 = [{"dmt": dmt_pad[c * NT_PAD:(c + 1) * NT_PAD], "v2r": v2r, "ones": onesr}
           for c in range(NCORES)]
    res1 = _run(_cache["p1"], in1, "p1")

    # partition-major device layout: table position == shard row
    ts_table = np.empty((NT, 2), np.float32)
    for c in range(NCORES):
        ts_table[c * NT_SH:(c + 1) * NT_SH] = res1.results[c]["tsloc"][:NT_SH]

    # ---- host glue: t lookup + shard/pad ----
    t_g = ts_table[ni, 0]                                # [NO, KC]
    cand_sh = np.zeros((NCORES, NO_PAD, KC, DIM), np.float32)
    pk_sh = np.zeros((NCORES, NO_PAD, KC), np.float32)
    for c in range(NCORES):
        cand_sh[c, :NO_SH] = Candidate[c * NO_SH:(c + 1) * NO_SH]
        pk_sh[c, :NO_SH] = t_g[c * NO_SH:(c + 1) * NO_SH]

    # ---- program 2: scores + argmin + heavy row selection ----
    if "p2" not in _cache:
        _cache["p2"] = build_p2()
    in2 = [{"cand": cand_sh[c], "pk": pk_sh[c], "v1r": v1r} for c in range(NCORES)]
    res2 = _run(_cache["p2"], in2, "p2")

    Cand_sel = np.empty((NO, K, DIM), np.float32)
    m = np.empty(NO, np.int64)
    for c in range(NCORES):
        rr = res2.results[c]
        Cand_sel[c * NO_SH:(c + 1) * NO_SH] = rr["oc"][:NO_SH]
        mc = np.rint(rr["om"]).astype(np.int64)          # [128, NTILE]
        m_full = mc.T.reshape(-1)                        # row n = t*128+p
        m[c * NO_SH:(c + 1) * NO_SH] = m_full[:NO_SH]

    # host finalization from the device argmin (index selection only)
    kk = np.arange(K)[None, :]
    sel = kk + (kk >= m[:, None])                        # [NO, K] selected k's
    nd_out = np.take_along_axis(neigh_dist, sel, axis=1)
    ni_out = np.take_along_axis(ni, sel, axis=1).astype(ni_in.dtype)
    s_g = ts_table[ni, 1].astype(np.float64)             # [NO, KC]
    b_rows = np.take_along_axis(s_g, m[:, None], axis=1)[:, 0]
    a_out = np.float32((s_g.sum() - b_rows.sum()) / (NO * K))
    b_out = np.float32(b_rows.sum() / NO)
    return (Cand_sel, nd_out, ni_out, a_out, b_out)


# revision 8
# speedup vs baseline: 1.8409x; 1.8409x over previous
"""Trainium2 Bass kernel for nn_AttentionLayer_77524159693050 (retrieval_knn).

Math reduction (verified against the reference):
  e[n,k] = eX[n] + eC[n,k]; top_k with k=KC-1 then sort == drop argmin_k eC.
  eC[n,k] = Candidate[n,k,:]@v1 + t[neigh_ind[n,k]],  t = data_m_train @ v2,
  where v = W @ A[OUT:,0], v1 = v[:DIM], v2 = v[DIM:].
  a_out/b_out only need s[j] = sum_d data_m_train[j,d] at neigh_ind positions.

Device program 1 (SPMD x8, data_m_train row-sharded): (t, s) pair tables.
Host glue: table lookup ts_table[neigh_ind] (the HW indirect DMA is
one-index-per-partition, so fine-grained gather is done host-side), plus
final index-selection of the small nd/ni outputs using the device argmin.
Device program 2 (SPMD x8, n row-sharded): e scores, argmin, and the heavy
10-of-11 candidate row selection (225MB read + 205MB write), spread across
the Vector, GpSimd and Scalar engines.
"""

import sys
import types
import numpy as np

sys.path.insert(0, "/opt/trn_rl_repo")

from concourse import bass, mybir, bacc  # noqa: E402
from concourse.tile import TileContext  # noqa: E402
from concourse.bass_utils import run_bass_kernel_spmd  # noqa: E402

f32 = mybir.dt.float32
i32 = mybir.dt.int32

NO, KC, DIM, NT, OUT, K = 20000, 11, 256, 100000, 128, 10
NCORES = 8
NO_SH = NO // NCORES          # 2500
NO_PAD = 2560                 # 20 tiles of 128
NTILE = NO_PAD // 128         # 20
NT_SH = NT // NCORES          # 12500
NT_BLK = 98                   # 128-row blocks per core
NT_PAD = 128 * NT_BLK         # 12544
P = 128

TRACE = {"enabled": False, "exec_ns": []}

_cache = {}


def _ntff_hook():
    try:
        from trn_agent_boot import trn_boot
        hook = trn_boot._ntff_profile_via_ctypes("/opt/axon/libaxon_pjrt.so")
        mod = types.ModuleType("antenv.axon_hooks")
        mod.get_axon_ntff_profile_hook = lambda: hook
        sys.modules["antenv.axon_hooks"] = mod
        return True
    except Exception:
        return False


def build_p1():
    nc = bacc.Bacc()
    dmt_d = nc.declare_dram_parameter("dmt", [NT_PAD, DIM], f32, isOutput=False)
    v2r_d = nc.declare_dram_parameter("v2r", [P, DIM], f32, isOutput=False)
    ones_d = nc.declare_dram_parameter("ones", [P, DIM], f32, isOutput=False)
    tsloc_d = nc.declare_dram_parameter("tsloc", [NT_PAD, 2], f32, isOutput=True)

    GB = 14            # 256-col blocks per DMA group
    NG = NT_BLK // GB  # 7
    NACT = 10          # s-blocks per group handled by ACT (rest on DVE)

    # partition-major: partition p holds shard rows [p*98, (p+1)*98)
    dmt_v = dmt_d[:].rearrange("(p i) d -> p i d", i=NT_BLK)      # [128, 98, 256]
    tsloc_v = tsloc_d[:].rearrange("(p i) c -> p i c", i=NT_BLK)  # [128, 98, 2]
    mult, add = mybir.AluOpType.mult, mybir.AluOpType.add

    with TileContext(nc) as tc:
        with (
            tc.tile_pool(name="io", bufs=3) as io_pool,
            tc.tile_pool(name="scr", bufs=2) as scr_pool,
            tc.tile_pool(name="acc", bufs=1) as acc_pool,
        ):
            v2r_t = acc_pool.tile([P, DIM], f32)
            nc.sync.dma_start(out=v2r_t[:], in_=v2r_d[:])
            ones_t = acc_pool.tile([P, DIM], f32)
            nc.sync.dma_start(out=ones_t[:], in_=ones_d[:])
            tsacc = acc_pool.tile([P, NT_BLK, 2], f32)
            for g in range(NG):
                in_t = io_pool.tile([P, GB, DIM], f32, tag="in")
                nc.sync.dma_start(out=in_t[:], in_=dmt_v[:, g * GB:(g + 1) * GB, :])
                for b in range(GB):
                    i = g * GB + b
                    scr_t = scr_pool.tile([P, DIM], f32, tag="scr")
                    nc.vector.scalar_tensor_tensor(
                        out=scr_t[:], in0=in_t[:, b, :], scalar=0.0,
                        in1=v2r_t[:], op0=add, op1=mult,
                        accum_out=tsacc[:, i, 0:1])
                    if b < NACT:
                        ascr_t = scr_pool.tile([P, DIM], f32, tag="ascr")
                        nc.scalar.activation(
                            out=ascr_t[:], in_=in_t[:, b, :],
                            func=mybir.ActivationFunctionType.Copy,
                            accum_out=tsacc[:, i, 1:2])
                    else:
                        sscr_t = scr_pool.tile([P, DIM], f32, tag="sscr")
                        nc.vector.scalar_tensor_tensor(
                            out=sscr_t[:], in0=in_t[:, b, :], scalar=0.0,
                            in1=ones_t[:], op0=add, op1=mult,
                            accum_out=tsacc[:, i, 1:2])
            nc.sync.dma_start(out=tsloc_v, in_=tsacc[:])
    nc.compile()
    return nc


def build_p2():
    nc = bacc.Bacc()
    cand_d = nc.declare_dram_parameter("cand", [NO_PAD, KC, DIM], f32, isOutput=False)
    pk_d = nc.declare_dram_parameter("pk", [NO_PAD, KC], f32, isOutput=False)
    v1r_d = nc.declare_dram_parameter("v1r", [P, DIM], f32, isOutput=False)
    oc_d = nc.declare_dram_parameter("oc", [NO_PAD, K, DIM], f32, isOutput=True)
    om_d = nc.declare_dram_parameter("om", [P, NTILE], f32, isOutput=True)

    mult, add, sub = mybir.AluOpType.mult, mybir.AluOpType.add, mybir.AluOpType.subtract
    is_eq, is_lt = mybir.AluOpType.is_equal, mybir.AluOpType.is_lt

    with TileContext(nc) as tc:
        with (
            tc.tile_pool(name="const", bufs=1) as cpool,
            tc.tile_pool(name="cio", bufs=4) as cio,
            tc.tile_pool(name="oio", bufs=4) as oio,
            tc.tile_pool(name="work", bufs=3) as wp,
            tc.tile_pool(name="small", bufs=4) as sp,
        ):
            v1r_t = cpool.tile([P, DIM], f32)
            nc.sync.dma_start(out=v1r_t[:], in_=v1r_d[:])
            kio_i = cpool.tile([P, KC], i32)
            nc.gpsimd.iota(kio_i[:], pattern=[[1, KC]], base=0, channel_multiplier=0)
            kio_f = cpool.tile([P, KC], f32)
            nc.vector.tensor_copy(kio_f[:], kio_i[:])
            mst_t = cpool.tile([P, NTILE], f32)

            for t in range(NTILE):
                r0 = t * P
                cand_t = cio.tile([P, KC, DIM], f32, tag="cand")
                nc.sync.dma_start(out=cand_t[:], in_=cand_d[r0:r0 + P])
                pk_t = sp.tile([P, KC], f32, tag="pk")
                nc.sync.dma_start(out=pk_t[:], in_=pk_d[r0:r0 + P])

                # e[p,k] = t_k + sum_d cand*v1  (c via fused mul+accum per k)
                c_t = sp.tile([P, KC], f32, tag="c")
                for k in range(KC):
                    scr_t = wp.tile([P, DIM], f32, tag="scr")
                    nc.vector.scalar_tensor_tensor(
                        out=scr_t[:], in0=cand_t[:, k, :], scalar=0.0,
                        in1=v1r_t[:], op0=add, op1=mult,
                        accum_out=c_t[:, k:k + 1])
                e_t = sp.tile([P, KC], f32, tag="e")
                nc.gpsimd.tensor_tensor(out=e_t[:], in0=c_t[:], in1=pk_t[:], op=add)

                # argmin over k
                mn_t = sp.tile([P, 1], f32, tag="mn")
                nc.vector.tensor_reduce(out=mn_t[:], in_=e_t[:],
                                        axis=mybir.AxisListType.X,
                                        op=mybir.AluOpType.min)
                scr11_t = sp.tile([P, KC], f32, tag="scr11")
                m_t = sp.tile([P, 1], f32, tag="m")
                nc.vector.scalar_tensor_tensor(
                    out=scr11_t[:], in0=e_t[:], scalar=mn_t[:], in1=kio_f[:],
                    op0=is_eq, op1=mult, accum_out=m_t[:])
                nc.scalar.activation(out=mst_t[:, t:t + 1], in_=m_t[:],
                                     func=mybir.ActivationFunctionType.Copy)
                mask_t = sp.tile([P, K], f32, tag="mask")
                nc.gpsimd.tensor_scalar(out=mask_t[:], in0=kio_f[:, 0:K],
                                        scalar1=m_t[:], scalar2=None, op0=is_lt)

                invm_t = sp.tile([P, K], f32, tag="invm")
                nc.scalar.activation(out=invm_t[:], in_=mask_t[:],
                                     func=mybir.ActivationFunctionType.Copy,
                                     scale=-1.0, bias=1.0)

                # selection: out_j = mask_j*A_j + (1-mask_j)*B_j
                mm_t = wp.tile([P, K, DIM], f32, tag="mm")
                for j in range(K):
                    nc.scalar.activation(out=mm_t[:, j, :], in_=cand_t[:, j + 1, :],
                                         func=mybir.ActivationFunctionType.Copy,
                                         scale=invm_t[:, j:j + 1])
                out_t = oio.tile([P, K, DIM], f32, tag="oc")
                for j in range(K):
                    nc.vector.scalar_tensor_tensor(
                        out=out_t[:, j, :], in0=cand_t[:, j, :],
                        scalar=mask_t[:, j:j + 1], in1=mm_t[:, j, :],
                        op0=mult, op1=add)
                nc.sync.dma_start(out=oc_d[r0:r0 + P], in_=out_t[:])

            nc.sync.dma_start(out=om_d[:], in_=mst_t[:])
    nc.compile()
    return nc


def _run(nc, in_maps, tag):
    if TRACE["enabled"]:
        import shutil
        _ntff_hook()
        shutil.rmtree(f"/tmp/knn_trace_{tag}", ignore_errors=True)
        res = run_bass_kernel_spmd(nc, in_maps, core_ids=list(range(NCORES)),
                                   trace=True, tmpdir=f"/tmp/knn_trace_{tag}")
        TRACE["exec_ns"].append((tag, res.exec_time_ns))
        return res
    return run_bass_kernel_spmd(nc, in_maps, core_ids=list(range(NCORES)))


def kernel(X, Candidate, neigh_dist, neigh_ind, data_m_train, data_m_batch,
           test, W, A, **_unused):
    Candidate = np.ascontiguousarray(np.asarray(Candidate, dtype=np.float32))
    neigh_dist = np.ascontiguousarray(np.asarray(neigh_dist, dtype=np.float32))
    ni_in = np.asarray(neigh_ind)
    ni = ni_in.astype(np.int64)
    dmt = np.ascontiguousarray(np.asarray(data_m_train, dtype=np.float32))
    W = np.asarray(W, dtype=np.float32)
    A = np.asarray(A, dtype=np.float32)

    v = (W.astype(np.float64) @ A[OUT:, 0].astype(np.float64)).astype(np.float32)
    v1, v2 = v[:DIM], v[DIM:]
    v1r = np.ascontiguousarray(np.broadcast_to(v1, (P, DIM)))
    v2r = np.ascontiguousarray(np.broadcast_to(v2, (P, DIM)))

    # ---- program 1: (t, s) pair tables ----
    if "p1" not in _cache:
        _cache["p1"] = build_p1()
    dmt_pad = np.zeros((NCORES * NT_PAD, DIM), np.float32)
    for c in range(NCORES):
        dmt_pad[c * NT_PAD:c * NT_PAD + NT_SH] = dmt[c * NT_SH:(c + 1) * NT_SH]
    onesr = np.ones((P, DIM), np.float32)
    in1 = [{"dmt": dmt_pad[c * NT_PAD:(c + 1) * NT_PAD], "v2r": v2r, "ones": onesr}
           for c in range(NCORES)]
    res1 = _run(_cache["p1"], in1, "p1")

    # partition-major device layout: table position == shard row
    ts_table = np.empty((NT, 2), np.float32)
    for c in range(NCORES):
        ts_table[c * NT_SH:(c + 1) * NT_SH] = res1.results[c]["tsloc"][:NT_SH]

    # ---- host glue: t lookup + shard/pad ----
    t_g = ts_table[ni, 0]                                # [NO, KC]
    cand_sh = np.zeros((NCORES, NO_PAD, KC, DIM), np.float32)
    pk_sh = np.zeros((NCORES, NO_PAD, KC), np.float32)
    for c in range(NCORES):
        cand_sh[c, :NO_SH] = Candidate[c * NO_SH:(c + 1) * NO_SH]
        pk_sh[c, :NO_SH] = t_g[c * NO_SH:(c + 1) * NO_SH]

    # ---- program 2: scores + argmin + heavy row selection ----
    if "p2" not in _cache:
        _cache["p2"] = build_p2()
    in2 = [{"cand": cand_sh[c], "pk": pk_sh[c], "v1r": v1r} for c in range(NCORES)]
    res2 = _run(_cache["p2"], in2, "p2")

    Cand_sel = np.empty((NO, K, DIM), np.float32)
    m = np.empty(NO, np.int64)
    for c in range(NCORES):
        rr = res2.results[c]
        Cand_sel[c * NO_SH:(c + 1) * NO_SH] = rr["oc"][:NO_SH]
        mc = np.rint(rr["om"]).astype(np.int64)          # [128, NTILE]
        m_full = mc.T.reshape(-1)                        # row n = t*128+p
        m[c * NO_SH:(c + 1) * NO_SH] = m_full[:NO_SH]

    # host finalization from the device argmin (index selection only)
    kk = np.arange(K)[None, :]
    sel = kk + (kk >= m[:, None])                        # [NO, K] selected k's
    nd_out = np.take_along_axis(neigh_dist, sel, axis=1)
    ni_out = np.take_along_axis(ni, sel, axis=1).astype(ni_in.dtype)
    s_g = ts_table[ni, 1].astype(np.float64)             # [NO, KC]
    b_rows = np.take_along_axis(s_g, m[:, None], axis=1)[:, 0]
    a_out = np.float32((s_g.sum() - b_rows.sum()) / (NO * K))
    b_out = np.float32(b_rows.sum() / NO)
    return (Cand_sel, nd_out, ni_out, a_out, b_out)


# revision 18
# speedup vs baseline: 2.3195x; 1.2600x over previous
"""Trainium2 Bass kernel for nn_AttentionLayer_77524159693050 (retrieval_knn).

Math reduction (verified against the reference):
  e[n,k] = eX[n] + eC[n,k]; top_k with k=KC-1 then sort == drop argmin_k eC.
  eC[n,k] = Candidate[n,k,:]@v1 + t[neigh_ind[n,k]],  t = data_m_train @ v2,
  where v = W @ A[OUT:,0], v1 = v[:DIM], v2 = v[DIM:].
  a_out/b_out only need s[j] = sum_d data_m_train[j,d] at neigh_ind positions.

Device program 1 (SPMD x8, data_m_train row-sharded): (t, s) pair tables.
Host glue: table lookup ts_table[neigh_ind] (the HW indirect DMA is
one-index-per-partition, so fine-grained gather is done host-side), plus
final index-selection of the small nd/ni outputs using the device argmin.
Device program 2 (SPMD x8, n row-sharded): e scores, argmin, and the heavy
10-of-11 candidate row selection (225MB read + 205MB write), spread across
the Vector, GpSimd and Scalar engines.
"""

import sys
import types
import numpy as np

sys.path.insert(0, "/opt/trn_rl_repo")

from concourse import bass, mybir, bacc  # noqa: E402
from concourse.tile import TileContext  # noqa: E402
from concourse.bass_utils import run_bass_kernel_spmd  # noqa: E402

f32 = mybir.dt.float32
i32 = mybir.dt.int32

NO, KC, DIM, NT, OUT, K = 20000, 11, 256, 100000, 128, 10
NCORES = 8
NO_SH = NO // NCORES          # 2500
NO_PAD = 2560                 # 20 tiles of 128
NTILE = NO_PAD // 128         # 20
NT_SH = NT // NCORES          # 12500
NT_BLK = 98                   # 128-row blocks per core
NT_PAD = 128 * NT_BLK         # 12544
P = 128

TRACE = {"enabled": False, "exec_ns": []}

_cache = {}


def _ntff_hook():
    try:
        from trn_agent_boot import trn_boot
        hook = trn_boot._ntff_profile_via_ctypes("/opt/axon/libaxon_pjrt.so")
        mod = types.ModuleType("antenv.axon_hooks")
        mod.get_axon_ntff_profile_hook = lambda: hook
        sys.modules["antenv.axon_hooks"] = mod
        return True
    except Exception:
        return False


def build_p1():
    nc = bacc.Bacc()
    dmt_d = nc.declare_dram_parameter("dmt", [NT_PAD, DIM], f32, isOutput=False)
    v2r_d = nc.declare_dram_parameter("v2r", [P, DIM], f32, isOutput=False)
    ones_d = nc.declare_dram_parameter("ones", [P, DIM], f32, isOutput=False)
    tsloc_d = nc.declare_dram_parameter("tsloc", [NT_PAD, 2], f32, isOutput=True)

    GB = 7             # 256-col blocks per DMA group
    NG = NT_BLK // GB  # 14
    NACT = 5           # s-blocks per group handled by ACT (rest on DVE)

    # partition-major: partition p holds shard rows [p*98, (p+1)*98)
    dmt_v = dmt_d[:].rearrange("(p i) d -> p i d", i=NT_BLK)      # [128, 98, 256]
    tsloc_v = tsloc_d[:].rearrange("(p i) c -> p i c", i=NT_BLK)  # [128, 98, 2]
    mult, add = mybir.AluOpType.mult, mybir.AluOpType.add

    with TileContext(nc) as tc:
        with (
            tc.tile_pool(name="io", bufs=4) as io_pool,
            tc.tile_pool(name="scr", bufs=3) as scr_pool,
            tc.tile_pool(name="acc", bufs=1) as acc_pool,
        ):
            v2r_t = acc_pool.tile([P, DIM], f32)
            nc.sync.dma_start(out=v2r_t[:], in_=v2r_d[:])
            ones_t = acc_pool.tile([P, DIM], f32)
            nc.sync.dma_start(out=ones_t[:], in_=ones_d[:])
            tsacc = acc_pool.tile([P, NT_BLK, 2], f32)
            for g in range(NG):
                in_t = io_pool.tile([P, GB, DIM], f32, tag="in")
                nc.sync.dma_start(out=in_t[:], in_=dmt_v[:, g * GB:(g + 1) * GB, :])
                for b in range(GB):
                    i = g * GB + b
                    scr_t = scr_pool.tile([P, DIM], f32, tag="scr")
                    nc.vector.scalar_tensor_tensor(
                        out=scr_t[:], in0=in_t[:, b, :], scalar=0.0,
                        in1=v2r_t[:], op0=add, op1=mult,
                        accum_out=tsacc[:, i, 0:1])
                    if b < NACT:
                        ascr_t = scr_pool.tile([P, DIM], f32, tag="ascr")
                        nc.scalar.activation(
                            out=ascr_t[:], in_=in_t[:, b, :],
                            func=mybir.ActivationFunctionType.Copy,
                            accum_out=tsacc[:, i, 1:2])
                    else:
                        sscr_t = scr_pool.tile([P, DIM], f32, tag="sscr")
                        nc.vector.scalar_tensor_tensor(
                            out=sscr_t[:], in0=in_t[:, b, :], scalar=0.0,
                            in1=ones_t[:], op0=add, op1=mult,
                            accum_out=tsacc[:, i, 1:2])
            nc.sync.dma_start(out=tsloc_v, in_=tsacc[:])
    nc.compile()
    return nc


def build_p2():
    nc = bacc.Bacc()
    cand_d = nc.declare_dram_parameter("cand", [NO_PAD, KC, DIM], f32, isOutput=False)
    pk_d = nc.declare_dram_parameter("pk", [NO_PAD, KC], f32, isOutput=False)
    v1r_d = nc.declare_dram_parameter("v1r", [P, DIM], f32, isOutput=False)
    oc_d = nc.declare_dram_parameter("oc", [NO_PAD, K, DIM], f32, isOutput=True)
    om_d = nc.declare_dram_parameter("om", [P, NTILE], f32, isOutput=True)

    mult, add, sub = mybir.AluOpType.mult, mybir.AluOpType.add, mybir.AluOpType.subtract
    is_eq, is_lt = mybir.AluOpType.is_equal, mybir.AluOpType.is_lt

    with TileContext(nc) as tc:
        with (
            tc.tile_pool(name="const", bufs=1) as cpool,
            tc.tile_pool(name="cio", bufs=5) as cio,
            tc.tile_pool(name="oio", bufs=4) as oio,
            tc.tile_pool(name="work", bufs=4) as wp,
            tc.tile_pool(name="small", bufs=6) as sp,
        ):
            v1r_t = cpool.tile([P, DIM], f32)
            nc.sync.dma_start(out=v1r_t[:], in_=v1r_d[:])
            kio_i = cpool.tile([P, KC], i32)
            nc.gpsimd.iota(kio_i[:], pattern=[[1, KC]], base=0, channel_multiplier=0)
            kio_f = cpool.tile([P, KC], f32)
            nc.vector.tensor_copy(kio_f[:], kio_i[:])
            mst_t = cpool.tile([P, NTILE], f32)

            def emit_blend(cand_t, mask_t, mm_t, r0):
                out_t = oio.tile([P, K, DIM], f32, tag="oc")
                for j in range(K):
                    nc.vector.scalar_tensor_tensor(
                        out=out_t[:, j, :], in0=cand_t[:, j, :],
                        scalar=mask_t[:, j:j + 1], in1=mm_t[:, j, :],
                        op0=mult, op1=add)
                nc.sync.dma_start(out=oc_d[r0:r0 + P], in_=out_t[:])

            pending = []
            for t in range(NTILE):
                r0 = t * P
                cand_t = cio.tile([P, KC, DIM], f32, tag="cand")
                nc.sync.dma_start(out=cand_t[:], in_=cand_d[r0:r0 + P])
                pk_t = sp.tile([P, KC], f32, tag="pk")
                nc.sync.dma_start(out=pk_t[:], in_=pk_d[r0:r0 + P])

                # e[p,k] = t_k + sum_d cand*v1  (c via fused mul+accum per k)
                c_t = sp.tile([P, KC], f32, tag="c")
                for k in range(KC):
                    scr_t = wp.tile([P, DIM], f32, tag="scr")
                    nc.vector.scalar_tensor_tensor(
                        out=scr_t[:], in0=cand_t[:, k, :], scalar=0.0,
                        in1=v1r_t[:], op0=add, op1=mult,
                        accum_out=c_t[:, k:k + 1])
                e_t = sp.tile([P, KC], f32, tag="e")
                nc.vector.tensor_tensor(out=e_t[:], in0=c_t[:], in1=pk_t[:], op=add)

                # argmin over k
                mn_t = sp.tile([P, 1], f32, tag="mn")
                nc.vector.tensor_reduce(out=mn_t[:], in_=e_t[:],
                                        axis=mybir.AxisListType.X,
                                        op=mybir.AluOpType.min)
                scr11_t = sp.tile([P, KC], f32, tag="scr11")
                m_t = sp.tile([P, 1], f32, tag="m")
                nc.vector.scalar_tensor_tensor(
                    out=scr11_t[:], in0=e_t[:], scalar=mn_t[:], in1=kio_f[:],
                    op0=is_eq, op1=mult, accum_out=m_t[:])
                nc.scalar.activation(out=mst_t[:, t:t + 1], in_=m_t[:],
                                     func=mybir.ActivationFunctionType.Copy)
                mask_t = sp.tile([P, K], f32, tag="mask")
                nc.vector.tensor_scalar(out=mask_t[:], in0=kio_f[:, 0:K],
                                        scalar1=m_t[:], scalar2=None, op0=is_lt)

                invm_t = sp.tile([P, K], f32, tag="invm")
                nc.scalar.activation(out=invm_t[:], in_=mask_t[:],
                                     func=mybir.ActivationFunctionType.Copy,
                                     scale=-1.0, bias=1.0)

                # selection: out_j = mask_j*A_j + (1-mask_j)*B_j
                mm_t = wp.tile([P, K, DIM], f32, tag="mm")
                for j in range(K):
                    nc.scalar.activation(out=mm_t[:, j, :], in_=cand_t[:, j + 1, :],
                                         func=mybir.ActivationFunctionType.Copy,
                                         scale=invm_t[:, j:j + 1])
                if len(pending) >= 2:
                    emit_blend(*pending.pop(0))
                pending.append((cand_t, mask_t, mm_t, r0))

            for args in pending:
                emit_blend(*args)
            nc.sync.dma_start(out=om_d[:], in_=mst_t[:])
    nc.compile()
    return nc


def _run(nc, in_maps, tag):
    if TRACE["enabled"]:
        import shutil
        _ntff_hook()
        shutil.rmtree(f"/tmp/knn_trace_{tag}", ignore_errors=True)
        res = run_bass_kernel_spmd(nc, in_maps, core_ids=list(range(NCORES)),
                                   trace=True, tmpdir=f"/tmp/knn_trace_{tag}")
        TRACE["exec_ns"].append((tag, res.exec_time_ns))
        return res
    return run_bass_kernel_spmd(nc, in_maps, core_ids=list(range(NCORES)))


def kernel(X, Candidate, neigh_dist, neigh_ind, data_m_train, data_m_batch,
           test, W, A, **_unused):
    Candidate = np.ascontiguousarray(np.asarray(Candidate, dtype=np.float32))
    neigh_dist = np.ascontiguousarray(np.asarray(neigh_dist, dtype=np.float32))
    ni_in = np.asarray(neigh_ind)
    ni = ni_in.astype(np.int64)
    dmt = np.ascontiguousarray(np.asarray(data_m_train, dtype=np.float32))
    W = np.asarray(W, dtype=np.float32)
    A = np.asarray(A, dtype=np.float32)

    v = (W.astype(np.float64) @ A[OUT:, 0].astype(np.float64)).astype(np.float32)
    v1, v2 = v[:DIM], v[DIM:]
    v1r = np.ascontiguousarray(np.broadcast_to(v1, (P, DIM)))
    v2r = np.ascontiguousarray(np.broadcast_to(v2, (P, DIM)))

    # ---- program 1: (t, s) pair tables ----
    if "p1" not in _cache:
        _cache["p1"] = build_p1()
    dmt_pad = np.zeros((NCORES * NT_PAD, DIM), np.float32)
    for c in range(NCORES):
        dmt_pad[c * NT_PAD:c * NT_PAD + NT_SH] = dmt[c * NT_SH:(c + 1) * NT_SH]
    onesr = np.ones((P, DIM), np.float32)
    in1 = [{"dmt": dmt_pad[c * NT_PAD:(c + 1) * NT_PAD], "v2r": v2r, "ones": onesr}
           for c in range(NCORES)]
    res1 = _run(_cache["p1"], in1, "p1")

    # partition-major device layout: table position == shard row
    ts_table = np.empty((NT, 2), np.float32)
    for c in range(NCORES):
        ts_table[c * NT_SH:(c + 1) * NT_SH] = res1.results[c]["tsloc"][:NT_SH]

    # ---- host glue: t lookup + shard/pad ----
    t_g = ts_table[ni, 0]                                # [NO, KC]
    cand_sh = np.zeros((NCORES, NO_PAD, KC, DIM), np.float32)
    pk_sh = np.zeros((NCORES, NO_PAD, KC), np.float32)
    for c in range(NCORES):
        cand_sh[c, :NO_SH] = Candidate[c * NO_SH:(c + 1) * NO_SH]
        pk_sh[c, :NO_SH] = t_g[c * NO_SH:(c + 1) * NO_SH]

    # ---- program 2: scores + argmin + heavy row selection ----
    if "p2" not in _cache:
        _cache["p2"] = build_p2()
    in2 = [{"cand": cand_sh[c], "pk": pk_sh[c], "v1r": v1r} for c in range(NCORES)]
    res2 = _run(_cache["p2"], in2, "p2")

    Cand_sel = np.empty((NO, K, DIM), np.float32)
    m = np.empty(NO, np.int64)
    for c in range(NCORES):
        rr = res2.results[c]
        Cand_sel[c * NO_SH:(c + 1) * NO_SH] = rr["oc"][:NO_SH]
        mc = np.rint(rr["om"]).astype(np.int64)          # [128, NTILE]
        m_full = mc.T.reshape(-1)                        # row n = t*128+p
        m[c * NO_SH:(c + 1) * NO_SH] = m_full[:NO_SH]

    # host finalization from the device argmin (index selection only)
    kk = np.arange(K)[None, :]
    sel = kk + (kk >= m[:, None])                        # [NO, K] selected k's
    nd_out = np.take_along_axis(neigh_dist, sel, axis=1)
    ni_out = np.take_along_axis(ni, sel, axis=1).astype(ni_in.dtype)
    s_g = ts_table[ni, 1].astype(np.float64)             # [NO, KC]
    b_rows = np.take_along_axis(s_g, m[:, None], axis=1)[:, 0]
    a_out = np.float32((s_g.sum() - b_rows.sum()) / (NO * K))
    b_out = np.float32(b_rows.sum() / NO)
    return (Cand_sel, nd_out, ni_out, a_out, b_out)


# revision 20
# speedup vs baseline: 2.3641x; 1.0192x over previous
"""Trainium2 Bass kernel for nn_AttentionLayer_77524159693050 (retrieval_knn).

Math reduction (verified against the reference):
  e[n,k] = eX[n] + eC[n,k]; top_k with k=KC-1 then sort == drop argmin_k eC.
  eC[n,k] = Candidate[n,k,:]@v1 + t[neigh_ind[n,k]],  t = data_m_train @ v2,
  where v = W @ A[OUT:,0], v1 = v[:DIM], v2 = v[DIM:].
  a_out/b_out only need s[j] = sum_d data_m_train[j,d] at neigh_ind positions.

Device program 1 (SPMD x8, data_m_train row-sharded): (t, s) pair tables.
Host glue: table lookup ts_table[neigh_ind] (the HW indirect DMA is
one-index-per-partition, so fine-grained gather is done host-side), plus
final index-selection of the small nd/ni outputs using the device argmin.
Device program 2 (SPMD x8, n row-sharded): e scores, argmin, and the heavy
10-of-11 candidate row selection (225MB read + 205MB write), spread across
the Vector, GpSimd and Scalar engines.
"""

import sys
import types
import numpy as np

sys.path.insert(0, "/opt/trn_rl_repo")

from concourse import bass, mybir, bacc  # noqa: E402
from concourse.tile import TileContext  # noqa: E402
from concourse.bass_utils import run_bass_kernel_spmd  # noqa: E402

f32 = mybir.dt.float32
i32 = mybir.dt.int32

NO, KC, DIM, NT, OUT, K = 20000, 11, 256, 100000, 128, 10
NCORES = 8
NO_SH = NO // NCORES          # 2500
NO_PAD = 2560                 # 20 tiles of 128
NTILE = NO_PAD // 128         # 20
NT_SH = NT // NCORES          # 12500
NT_BLK = 98                   # 128-row blocks per core
NT_PAD = 128 * NT_BLK         # 12544
P = 128

TRACE = {"enabled": False, "exec_ns": []}

_cache = {}


def _ntff_hook():
    try:
        from trn_agent_boot import trn_boot
        hook = trn_boot._ntff_profile_via_ctypes("/opt/axon/libaxon_pjrt.so")
        mod = types.ModuleType("antenv.axon_hooks")
        mod.get_axon_ntff_profile_hook = lambda: hook
        sys.modules["antenv.axon_hooks"] = mod
        return True
    except Exception:
        return False


def build_p1():
    nc = bacc.Bacc()
    dmt_d = nc.declare_dram_parameter("dmt", [NT_PAD, DIM], f32, isOutput=False)
    v2r_d = nc.declare_dram_parameter("v2r", [P, DIM], f32, isOutput=False)
    ones_d = nc.declare_dram_parameter("ones", [P, DIM], f32, isOutput=False)
    tsloc_d = nc.declare_dram_parameter("tsloc", [NT_PAD, 2], f32, isOutput=True)

    GB = 7             # 256-col blocks per DMA group
    NG = NT_BLK // GB  # 14
    NACT = 5           # s-blocks per group handled by ACT (rest on DVE)

    # partition-major: partition p holds shard rows [p*98, (p+1)*98)
    dmt_v = dmt_d[:].rearrange("(p i) d -> p i d", i=NT_BLK)      # [128, 98, 256]
    tsloc_v = tsloc_d[:].rearrange("(p i) c -> p i c", i=NT_BLK)  # [128, 98, 2]
    mult, add = mybir.AluOpType.mult, mybir.AluOpType.add

    with TileContext(nc) as tc:
        with (
            tc.tile_pool(name="io", bufs=4) as io_pool,
            tc.tile_pool(name="scr", bufs=3) as scr_pool,
            tc.tile_pool(name="acc", bufs=1) as acc_pool,
        ):
            v2r_t = acc_pool.tile([P, DIM], f32)
            nc.sync.dma_start(out=v2r_t[:], in_=v2r_d[:])
            ones_t = acc_pool.tile([P, DIM], f32)
            nc.sync.dma_start(out=ones_t[:], in_=ones_d[:])
            tsacc = acc_pool.tile([P, NT_BLK, 2], f32)
            for g in range(NG):
                in_t = io_pool.tile([P, GB, DIM], f32, tag="in")
                nc.sync.dma_start(out=in_t[:], in_=dmt_v[:, g * GB:(g + 1) * GB, :])
                for b in range(GB):
                    i = g * GB + b
                    scr_t = scr_pool.tile([P, DIM], f32, tag="scr")
                    nc.vector.scalar_tensor_tensor(
                        out=scr_t[:], in0=in_t[:, b, :], scalar=0.0,
                        in1=v2r_t[:], op0=add, op1=mult,
                        accum_out=tsacc[:, i, 0:1])
                    if b < NACT:
                        ascr_t = scr_pool.tile([P, DIM], f32, tag="ascr")
                        nc.scalar.activation(
                            out=ascr_t[:], in_=in_t[:, b, :],
                            func=mybir.ActivationFunctionType.Copy,
                            accum_out=tsacc[:, i, 1:2])
                    else:
                        sscr_t = scr_pool.tile([P, DIM], f32, tag="sscr")
                        nc.vector.scalar_tensor_tensor(
                            out=sscr_t[:], in0=in_t[:, b, :], scalar=0.0,
                            in1=ones_t[:], op0=add, op1=mult,
                            accum_out=tsacc[:, i, 1:2])
            nc.sync.dma_start(out=tsloc_v, in_=tsacc[:])
    nc.compile()
    return nc


def build_p2():
    nc = bacc.Bacc()
    cand_d = nc.declare_dram_parameter("cand", [NO_PAD, KC, DIM], f32, isOutput=False)
    pk_d = nc.declare_dram_parameter("pk", [NO_PAD, KC], f32, isOutput=False)
    v1r_d = nc.declare_dram_parameter("v1r", [P, DIM], f32, isOutput=False)
    oc_d = nc.declare_dram_parameter("oc", [NO_PAD, K, DIM], f32, isOutput=True)
    om_d = nc.declare_dram_parameter("om", [P, NTILE], f32, isOutput=True)

    mult, add, sub = mybir.AluOpType.mult, mybir.AluOpType.add, mybir.AluOpType.subtract
    is_eq, is_lt = mybir.AluOpType.is_equal, mybir.AluOpType.is_lt

    with TileContext(nc) as tc:
        with (
            tc.tile_pool(name="const", bufs=1) as cpool,
            tc.tile_pool(name="cio", bufs=5) as cio,
            tc.tile_pool(name="oio", bufs=4) as oio,
            tc.tile_pool(name="work", bufs=4) as wp,
            tc.tile_pool(name="small", bufs=6) as sp,
        ):
            v1r_t = cpool.tile([P, DIM], f32)
            nc.sync.dma_start(out=v1r_t[:], in_=v1r_d[:])
            kio_i = cpool.tile([P, KC], i32)
            nc.gpsimd.iota(kio_i[:], pattern=[[1, KC]], base=0, channel_multiplier=0)
            kio_f = cpool.tile([P, KC], f32)
            nc.vector.tensor_copy(kio_f[:], kio_i[:])
            mst_t = cpool.tile([P, NTILE], f32)

            def emit_blend(cand_t, mask_t, mm_t, r0):
                out_t = oio.tile([P, K, DIM], f32, tag="oc")
                for j in range(K):
                    nc.vector.scalar_tensor_tensor(
                        out=out_t[:, j, :], in0=cand_t[:, j, :],
                        scalar=mask_t[:, j:j + 1], in1=mm_t[:, j, :],
                        op0=mult, op1=add)
                nc.scalar.dma_start(out=oc_d[r0:r0 + P], in_=out_t[:])

            pending = []
            for t in range(NTILE):
                r0 = t * P
                cand_t = cio.tile([P, KC, DIM], f32, tag="cand")
                if t == 0:
                    nc.sync.dma_start(out=cand_t[:, 0:4, :],
                                      in_=cand_d[r0:r0 + P, 0:4, :])
                    nc.sync.dma_start(out=cand_t[:, 4:KC, :],
                                      in_=cand_d[r0:r0 + P, 4:KC, :])
                else:
                    nc.sync.dma_start(out=cand_t[:], in_=cand_d[r0:r0 + P])
                pk_t = sp.tile([P, KC], f32, tag="pk")
                nc.sync.dma_start(out=pk_t[:], in_=pk_d[r0:r0 + P])

                # e[p,k] = t_k + sum_d cand*v1  (c via fused mul+accum per k)
                c_t = sp.tile([P, KC], f32, tag="c")
                for k in range(KC):
                    scr_t = wp.tile([P, DIM], f32, tag="scr")
                    nc.vector.scalar_tensor_tensor(
                        out=scr_t[:], in0=cand_t[:, k, :], scalar=0.0,
                        in1=v1r_t[:], op0=add, op1=mult,
                        accum_out=c_t[:, k:k + 1])
                e_t = sp.tile([P, KC], f32, tag="e")
                nc.vector.tensor_tensor(out=e_t[:], in0=c_t[:], in1=pk_t[:], op=add)

                # argmin over k
                mn_t = sp.tile([P, 1], f32, tag="mn")
                nc.vector.tensor_reduce(out=mn_t[:], in_=e_t[:],
                                        axis=mybir.AxisListType.X,
                                        op=mybir.AluOpType.min)
                scr11_t = sp.tile([P, KC], f32, tag="scr11")
                m_t = sp.tile([P, 1], f32, tag="m")
                nc.vector.scalar_tensor_tensor(
                    out=scr11_t[:], in0=e_t[:], scalar=mn_t[:], in1=kio_f[:],
                    op0=is_eq, op1=mult, accum_out=m_t[:])
                nc.scalar.activation(out=mst_t[:, t:t + 1], in_=m_t[:],
                                     func=mybir.ActivationFunctionType.Copy)
                mask_t = sp.tile([P, K], f32, tag="mask")
                nc.vector.tensor_scalar(out=mask_t[:], in0=kio_f[:, 0:K],
                                        scalar1=m_t[:], scalar2=None, op0=is_lt)

                invm_t = sp.tile([P, K], f32, tag="invm")
                nc.scalar.activation(out=invm_t[:], in_=mask_t[:],
                                     func=mybir.ActivationFunctionType.Copy,
                                     scale=-1.0, bias=1.0)

                # selection: out_j = mask_j*A_j + (1-mask_j)*B_j
                mm_t = wp.tile([P, K, DIM], f32, tag="mm")
                for j in range(K):
                    nc.scalar.activation(out=mm_t[:, j, :], in_=cand_t[:, j + 1, :],
                                         func=mybir.ActivationFunctionType.Copy,
                                         scale=invm_t[:, j:j + 1])
                if len(pending) >= 2:
                    emit_blend(*pending.pop(0))
                pending.append((cand_t, mask_t, mm_t, r0))

            for args in pending:
                emit_blend(*args)
            nc.sync.dma_start(out=om_d[:], in_=mst_t[:])
    nc.compile()
    return nc


def _run(nc, in_maps, tag):
    if TRACE["enabled"]:
        import shutil
        _ntff_hook()
        shutil.rmtree(f"/tmp/knn_trace_{tag}", ignore_errors=True)
        res = run_bass_kernel_spmd(nc, in_maps, core_ids=list(range(NCORES)),
                                   trace=True, tmpdir=f"/tmp/knn_trace_{tag}")
        TRACE["exec_ns"].append((tag, res.exec_time_ns))
        return res
    return run_bass_kernel_spmd(nc, in_maps, core_ids=list(range(NCORES)))


def kernel(X, Candidate, neigh_dist, neigh_ind, data_m_train, data_m_batch,
           test, W, A, **_unused):
    Candidate = np.ascontiguousarray(np.asarray(Candidate, dtype=np.float32))
    neigh_dist = np.ascontiguousarray(np.asarray(neigh_dist, dtype=np.float32))
    ni_in = np.asarray(neigh_ind)
    ni = ni_in.astype(np.int64)
    dmt = np.ascontiguousarray(np.asarray(data_m_train, dtype=np.float32))
    W = np.asarray(W, dtype=np.float32)
    A = np.asarray(A, dtype=np.float32)

    v = (W.astype(np.float64) @ A[OUT:, 0].astype(np.float64)).astype(np.float32)
    v1, v2 = v[:DIM], v[DIM:]
    v1r = np.ascontiguousarray(np.broadcast_to(v1, (P, DIM)))
    v2r = np.ascontiguousarray(np.broadcast_to(v2, (P, DIM)))

    # ---- program 1: (t, s) pair tables ----
    if "p1" not in _cache:
        _cache["p1"] = build_p1()
    dmt_pad = np.zeros((NCORES * NT_PAD, DIM), np.float32)
    for c in range(NCORES):
        dmt_pad[c * NT_PAD:c * NT_PAD + NT_SH] = dmt[c * NT_SH:(c + 1) * NT_SH]
    onesr = np.ones((P, DIM), np.float32)
    in1 = [{"dmt": dmt_pad[c * NT_PAD:(c + 1) * NT_PAD], "v2r": v2r, "ones": onesr}
           for c in range(NCORES)]
    res1 = _run(_cache["p1"], in1, "p1")

    # partition-major device layout: table position == shard row
    ts_table = np.empty((NT, 2), np.float32)
    for c in range(NCORES):
        ts_table[c * NT_SH:(c + 1) * NT_SH] = res1.results[c]["tsloc"][:NT_SH]

    # ---- host glue: t lookup + shard/pad ----
    t_g = ts_table[ni, 0]                                # [NO, KC]
    cand_sh = np.zeros((NCORES, NO_PAD, KC, DIM), np.float32)
    pk_sh = np.zeros((NCORES, NO_PAD, KC), np.float32)
    for c in range(NCORES):
        cand_sh[c, :NO_SH] = Candidate[c * NO_SH:(c + 1) * NO_SH]
        pk_sh[c, :NO_SH] = t_g[c * NO_SH:(c + 1) * NO_SH]

    # ---- program 2: scores + argmin + heavy row selection ----
    if "p2" not in _cache:
        _cache["p2"] = build_p2()
    in2 = [{"cand": cand_sh[c], "pk": pk_sh[c], "v1r": v1r} for c in range(NCORES)]
    res2 = _run(_cache["p2"], in2, "p2")

    Cand_sel = np.empty((NO, K, DIM), np.float32)
    m = np.empty(NO, np.int64)
    for c in range(NCORES):
        rr = res2.results[c]
        Cand_sel[c * NO_SH:(c + 1) * NO_SH] = rr["oc"][:NO_SH]
        mc = np.rint(rr["om"]).astype(np.int64)          # [128, NTILE]
        m_full = mc.T.reshape(-1)                        # row n = t*128+p
        m[c * NO_SH:(c + 1) * NO_SH] = m_full[:NO_SH]

    # host finalization from the device argmin (index selection only)
    kk = np.arange(K)[None, :]
    sel = kk + (kk >= m[:, None])                        # [NO, K] selected k's
    nd_out = np.take_along_axis(neigh_dist, sel, axis=1)
    ni_out = np.take_along_axis(ni, sel, axis=1).astype(ni_in.dtype)
    s_g = ts_table[ni, 1].astype(np.float64)             # [NO, KC]
    b_rows = np.take_along_axis(s_g, m[:, None], axis=1)[:, 0]
    a_out = np.float32((s_g.sum() - b_rows.sum()) / (NO * K))
    b_out = np.float32(b_rows.sum() / NO)
    return (Cand_sel, nd_out, ni_out, a_out, b_out)


# revision 21
# speedup vs baseline: 2.5625x; 1.0839x over previous
"""Trainium2 Bass kernel for nn_AttentionLayer_77524159693050 (retrieval_knn).

Math reduction (verified against the reference):
  e[n,k] = eX[n] + eC[n,k]; top_k with k=KC-1 then sort == drop argmin_k eC.
  eC[n,k] = Candidate[n,k,:]@v1 + t[neigh_ind[n,k]],  t = data_m_train @ v2,
  where v = W @ A[OUT:,0], v1 = v[:DIM], v2 = v[DIM:].
  a_out/b_out only need s[j] = sum_d data_m_train[j,d] at neigh_ind positions.

Device program 1 (SPMD x8, data_m_train row-sharded): (t, s) pair tables.
Host glue: table lookup ts_table[neigh_ind] (the HW indirect DMA is
one-index-per-partition, so fine-grained gather is done host-side), plus
final index-selection of the small nd/ni outputs using the device argmin.
Device program 2 (SPMD x8, n row-sharded): e scores, argmin, and the heavy
10-of-11 candidate row selection (225MB read + 205MB write), spread across
the Vector, GpSimd and Scalar engines.
"""

import sys
import types
import numpy as np

sys.path.insert(0, "/opt/trn_rl_repo")

from concourse import bass, mybir, bacc  # noqa: E402
from concourse.tile import TileContext  # noqa: E402
from concourse.bass_utils import run_bass_kernel_spmd  # noqa: E402

f32 = mybir.dt.float32
i32 = mybir.dt.int32

NO, KC, DIM, NT, OUT, K = 20000, 11, 256, 100000, 128, 10
NCORES = 8
NO_SH = NO // NCORES          # 2500
NO_PAD = 2560                 # 20 tiles of 128
NTILE = NO_PAD // 128         # 20
NT_SH = NT // NCORES          # 12500
NT_BLK = 98                   # 128-row blocks per core
NT_PAD = 128 * NT_BLK         # 12544
P = 128

TRACE = {"enabled": False, "exec_ns": []}

_cache = {}


def _ntff_hook():
    try:
        from trn_agent_boot import trn_boot
        hook = trn_boot._ntff_profile_via_ctypes("/opt/axon/libaxon_pjrt.so")
        mod = types.ModuleType("antenv.axon_hooks")
        mod.get_axon_ntff_profile_hook = lambda: hook
        sys.modules["antenv.axon_hooks"] = mod
        return True
    except Exception:
        return False


def build_p1():
    nc = bacc.Bacc()
    dmt_d = nc.declare_dram_parameter("dmt", [NT_PAD, DIM], f32, isOutput=False)
    v2r_d = nc.declare_dram_parameter("v2r", [P, DIM], f32, isOutput=False)
    ones_d = nc.declare_dram_parameter("ones", [P, DIM], f32, isOutput=False)
    tsloc_d = nc.declare_dram_parameter("tsloc", [NT_PAD, 2], f32, isOutput=True)

    GB = 7             # 256-col blocks per DMA group
    NG = NT_BLK // GB  # 14
    NACT = 5           # s-blocks per group handled by ACT (rest on DVE)

    # partition-major: partition p holds shard rows [p*98, (p+1)*98)
    dmt_v = dmt_d[:].rearrange("(p i) d -> p i d", i=NT_BLK)      # [128, 98, 256]
    tsloc_v = tsloc_d[:].rearrange("(p i) c -> p i c", i=NT_BLK)  # [128, 98, 2]
    mult, add = mybir.AluOpType.mult, mybir.AluOpType.add

    with TileContext(nc) as tc:
        with (
            tc.tile_pool(name="io", bufs=4) as io_pool,
            tc.tile_pool(name="scr", bufs=3) as scr_pool,
            tc.tile_pool(name="acc", bufs=1) as acc_pool,
        ):
            v2r_t = acc_pool.tile([P, DIM], f32)
            nc.sync.dma_start(out=v2r_t[:], in_=v2r_d[:])
            ones_t = acc_pool.tile([P, DIM], f32)
            nc.sync.dma_start(out=ones_t[:], in_=ones_d[:])
            tsacc = acc_pool.tile([P, NT_BLK, 2], f32)
            for g in range(NG):
                in_t = io_pool.tile([P, GB, DIM], f32, tag="in")
                if g == 0:
                    nc.sync.dma_start(out=in_t[:, 0:2, :], in_=dmt_v[:, 0:2, :])
                    nc.sync.dma_start(out=in_t[:, 2:GB, :], in_=dmt_v[:, 2:GB, :])
                else:
                    nc.sync.dma_start(out=in_t[:],
                                      in_=dmt_v[:, g * GB:(g + 1) * GB, :])
                for b in range(GB):
                    i = g * GB + b
                    scr_t = scr_pool.tile([P, DIM], f32, tag="scr")
                    nc.vector.scalar_tensor_tensor(
                        out=scr_t[:], in0=in_t[:, b, :], scalar=0.0,
                        in1=v2r_t[:], op0=add, op1=mult,
                        accum_out=tsacc[:, i, 0:1])
                    if b < NACT:
                        ascr_t = scr_pool.tile([P, DIM], f32, tag="ascr")
                        nc.scalar.activation(
                            out=ascr_t[:], in_=in_t[:, b, :],
                            func=mybir.ActivationFunctionType.Copy,
                            accum_out=tsacc[:, i, 1:2])
                    else:
                        sscr_t = scr_pool.tile([P, DIM], f32, tag="sscr")
                        nc.vector.scalar_tensor_tensor(
                            out=sscr_t[:], in0=in_t[:, b, :], scalar=0.0,
                            in1=ones_t[:], op0=add, op1=mult,
                            accum_out=tsacc[:, i, 1:2])
            nc.sync.dma_start(out=tsloc_v, in_=tsacc[:])
    nc.compile()
    return nc


def build_p2():
    nc = bacc.Bacc()
    cand_d = nc.declare_dram_parameter("cand", [NO_PAD, KC, DIM], f32, isOutput=False)
    pk_d = nc.declare_dram_parameter("pk", [NO_PAD, KC], f32, isOutput=False)
    v1r_d = nc.declare_dram_parameter("v1r", [P, DIM], f32, isOutput=False)
    oc_d = nc.declare_dram_parameter("oc", [NO_PAD, K, DIM], f32, isOutput=True)
    om_d = nc.declare_dram_parameter("om", [P, NTILE], f32, isOutput=True)

    mult, add, sub = mybir.AluOpType.mult, mybir.AluOpType.add, mybir.AluOpType.subtract
    is_eq, is_lt = mybir.AluOpType.is_equal, mybir.AluOpType.is_lt

    with TileContext(nc) as tc:
        with (
            tc.tile_pool(name="const", bufs=1) as cpool,
            tc.tile_pool(name="cio", bufs=6) as cio,
            tc.tile_pool(name="oio", bufs=4) as oio,
            tc.tile_pool(name="work", bufs=4) as wp,
            tc.tile_pool(name="small", bufs=6) as sp,
        ):
            v1r_t = cpool.tile([P, DIM], f32)
            nc.sync.dma_start(out=v1r_t[:], in_=v1r_d[:])
            kio_i = cpool.tile([P, KC], i32)
            nc.gpsimd.iota(kio_i[:], pattern=[[1, KC]], base=0, channel_multiplier=0)
            kio_f = cpool.tile([P, KC], f32)
            nc.vector.tensor_copy(kio_f[:], kio_i[:])
            mst_t = cpool.tile([P, NTILE], f32)

            def emit_blend(cand_t, mask_t, mm_t, r0):
                out_t = oio.tile([P, K, DIM], f32, tag="oc")
                for j in range(K):
                    nc.vector.scalar_tensor_tensor(
                        out=out_t[:, j, :], in0=cand_t[:, j, :],
                        scalar=mask_t[:, j:j + 1], in1=mm_t[:, j, :],
                        op0=mult, op1=add)
                nc.scalar.dma_start(out=oc_d[r0:r0 + P], in_=out_t[:])

            pending = []
            for t in range(NTILE):
                r0 = t * P
                cand_t = cio.tile([P, KC, DIM], f32, tag="cand")
                if t == 0:
                    nc.sync.dma_start(out=cand_t[:, 0:4, :],
                                      in_=cand_d[r0:r0 + P, 0:4, :])
                    nc.sync.dma_start(out=cand_t[:, 4:KC, :],
                                      in_=cand_d[r0:r0 + P, 4:KC, :])
                else:
                    nc.sync.dma_start(out=cand_t[:], in_=cand_d[r0:r0 + P])
                pk_t = sp.tile([P, KC], f32, tag="pk")
                nc.sync.dma_start(out=pk_t[:], in_=pk_d[r0:r0 + P])

                # e[p,k] = t_k + sum_d cand*v1  (c via fused mul+accum per k)
                c_t = sp.tile([P, KC], f32, tag="c")
                for k in range(KC):
                    scr_t = wp.tile([P, DIM], f32, tag="scr")
                    nc.vector.scalar_tensor_tensor(
                        out=scr_t[:], in0=cand_t[:, k, :], scalar=0.0,
                        in1=v1r_t[:], op0=add, op1=mult,
                        accum_out=c_t[:, k:k + 1])
                e_t = sp.tile([P, KC], f32, tag="e")
                nc.vector.tensor_tensor(out=e_t[:], in0=c_t[:], in1=pk_t[:], op=add)

                # argmin over k
                mn_t = sp.tile([P, 1], f32, tag="mn")
                nc.vector.tensor_reduce(out=mn_t[:], in_=e_t[:],
                                        axis=mybir.AxisListType.X,
                                        op=mybir.AluOpType.min)
                scr11_t = sp.tile([P, KC], f32, tag="scr11")
                m_t = sp.tile([P, 1], f32, tag="m")
                nc.vector.scalar_tensor_tensor(
                    out=scr11_t[:], in0=e_t[:], scalar=mn_t[:], in1=kio_f[:],
                    op0=is_eq, op1=mult, accum_out=m_t[:])
                nc.scalar.activation(out=mst_t[:, t:t + 1], in_=m_t[:],
                                     func=mybir.ActivationFunctionType.Copy)
                mask_t = sp.tile([P, K], f32, tag="mask")
                nc.vector.tensor_scalar(out=mask_t[:], in0=kio_f[:, 0:K],
                                        scalar1=m_t[:], scalar2=None, op0=is_lt)

                invm_t = sp.tile([P, K], f32, tag="invm")
                nc.scalar.activation(out=invm_t[:], in_=mask_t[:],
                                     func=mybir.ActivationFunctionType.Copy,
                                     scale=-1.0, bias=1.0)

                # selection: out_j = mask_j*A_j + (1-mask_j)*B_j
                mm_t = wp.tile([P, K, DIM], f32, tag="mm")
                for j in range(K):
                    nc.scalar.activation(out=mm_t[:, j, :], in_=cand_t[:, j + 1, :],
                                         func=mybir.ActivationFunctionType.Copy,
                                         scale=invm_t[:, j:j + 1])
                if len(pending) >= 2:
                    emit_blend(*pending.pop(0))
                pending.append((cand_t, mask_t, mm_t, r0))

            for args in pending:
                emit_blend(*args)
            nc.sync.dma_start(out=om_d[:], in_=mst_t[:])
    nc.compile()
    return nc


def _run(nc, in_maps, tag):
    if TRACE["enabled"]:
        import shutil
        _ntff_hook()
        shutil.rmtree(f"/tmp/knn_trace_{tag}", ignore_errors=True)
        res = run_bass_kernel_spmd(nc, in_maps, core_ids=list(range(NCORES)),
                                   trace=True, tmpdir=f"/tmp/knn_trace_{tag}")
        TRACE["exec_ns"].append((tag, res.exec_time_ns))
        return res
    return run_bass_kernel_spmd(nc, in_maps, core_ids=list(range(NCORES)))


def kernel(X, Candidate, neigh_dist, neigh_ind, data_m_train, data_m_batch,
           test, W, A, **_unused):
    Candidate = np.ascontiguousarray(np.asarray(Candidate, dtype=np.float32))
    neigh_dist = np.ascontiguousarray(np.asarray(neigh_dist, dtype=np.float32))
    ni_in = np.asarray(neigh_ind)
    ni = ni_in.astype(np.int64)
    dmt = np.ascontiguousarray(np.asarray(data_m_train, dtype=np.float32))
    W = np.asarray(W, dtype=np.float32)
    A = np.asarray(A, dtype=np.float32)

    v = (W.astype(np.float64) @ A[OUT:, 0].astype(np.float64)).astype(np.float32)
    v1, v2 = v[:DIM], v[DIM:]
    v1r = np.ascontiguousarray(np.broadcast_to(v1, (P, DIM)))
    v2r = np.ascontiguousarray(np.broadcast_to(v2, (P, DIM)))

    # ---- program 1: (t, s) pair tables ----
    if "p1" not in _cache:
        _cache["p1"] = build_p1()
    dmt_pad = np.zeros((NCORES * NT_PAD, DIM), np.float32)
    for c in range(NCORES):
        dmt_pad[c * NT_PAD:c * NT_PAD + NT_SH] = dmt[c * NT_SH:(c + 1) * NT_SH]
    onesr = np.ones((P, DIM), np.float32)
    in1 = [{"dmt": dmt_pad[c * NT_PAD:(c + 1) * NT_PAD], "v2r": v2r, "ones": onesr}
           for c in range(NCORES)]
    res1 = _run(_cache["p1"], in1, "p1")

    # partition-major device layout: table position == shard row
    ts_table = np.empty((NT, 2), np.float32)
    for c in range(NCORES):
        ts_table[c * NT_SH:(c + 1) * NT_SH] = res1.results[c]["tsloc"][:NT_SH]

    # ---- host glue: t lookup + shard/pad ----
    t_g = ts_table[ni, 0]                                # [NO, KC]
    cand_sh = np.zeros((NCORES, NO_PAD, KC, DIM), np.float32)
    pk_sh = np.zeros((NCORES, NO_PAD, KC), np.float32)
    for c in range(NCORES):
        cand_sh[c, :NO_SH] = Candidate[c * NO_SH:(c + 1) * NO_SH]
        pk_sh[c, :NO_SH] = t_g[c * NO_SH:(c + 1) * NO_SH]

    # ---- program 2: scores + argmin + heavy row selection ----
    if "p2" not in _cache:
        _cache["p2"] = build_p2()
    in2 = [{"cand": cand_sh[c], "pk": pk_sh[c], "v1r": v1r} for c in range(NCORES)]
    res2 = _run(_cache["p2"], in2, "p2")

    Cand_sel = np.empty((NO, K, DIM), np.float32)
    m = np.empty(NO, np.int64)
    for c in range(NCORES):
        rr = res2.results[c]
        Cand_sel[c * NO_SH:(c + 1) * NO_SH] = rr["oc"][:NO_SH]
        mc = np.rint(rr["om"]).astype(np.int64)          # [128, NTILE]
        m_full = mc.T.reshape(-1)                        # row n = t*128+p
        m[c * NO_SH:(c + 1) * NO_SH] = m_full[:NO_SH]

    # host finalization from the device argmin (index selection only)
    kk = np.arange(K)[None, :]
    sel = kk + (kk >= m[:, None])                        # [NO, K] selected k's
    nd_out = np.take_along_axis(neigh_dist, sel, axis=1)
    ni_out = np.take_along_axis(ni, sel, axis=1).astype(ni_in.dtype)
    s_g = ts_table[ni, 1].astype(np.float64)             # [NO, KC]
    b_rows = np.take_along_axis(s_g, m[:, None], axis=1)[:, 0]
    a_out = np.float32((s_g.sum() - b_rows.sum()) / (NO * K))
    b_out = np.float32(b_rows.sum() / NO)
    return (Cand_sel, nd_out, ni_out, a_out, b_out)
